# revision 39
# baseline (speedup 1.0000x reference)
"""Trainium2 Bass kernel for a 2-layer GCN with data-aware attention gate.

Math (per reference):
    src,dst = edges + self-loops; deg = bincount(dst); dinv = rsqrt(deg)
    norm = dinv[src]*dinv[dst]
    h1 = relu(segsum(norm * (x@W1)[src], dst) + b1)
    h2 = relu(segsum(norm * (h1@W2)[src], dst) + b2)
    out = h2 * sigmoid(h2@attn_w + attn_b)

Device strategy (8 NeuronCores, node/dst-sharded), v2:
    norm factorizes: agg[d] = dinv[d] * sum_{e->d} (dinv[s] * T[s]).
    Tables are plain bf16 (tolerance 2e-2 leaves plenty of headroom), rows
    pair-packed so each 256B dma_gather element holds two consecutive table
    rows; edges are bucketed by src-row parity so the wanted row sits at a
    static column offset of the gathered element.  Segment sums are one-hot
    selection-matrix matmuls accumulated d-major in PSUM (lhsT = gathered
    data, rhs = S), with the S matrices built in bulk on the vector engine
    via a broadcast is_equal against an iota tile.  Self-loops skip the
    gather entirely: one extra identity-selection matmul per window against
    the SBUF-resident local table shard (table rows already carry dinv_src).
    dinv_src is folded into x on the host; dinv_dst is applied in the
    flush via a host-replicated dinv row tile and a scalar-engine scale.
"""

import sys

import numpy as np

_CONC = "/opt/trn_rl_repo"
if _CONC not in sys.path:
    sys.path.insert(0, _CONC)

# ---------------------------------------------------------------------------
# configuration
# ---------------------------------------------------------------------------


class Cfg:
    def __init__(self, N=50000, DIN=128, DH=64, DOUT=32, NC=8, WPC=49, WPG=7):
        self.N, self.DIN, self.DH, self.DOUT = N, DIN, DH, DOUT
        self.NC, self.WPC, self.WPG = NC, WPC, WPG
        assert WPC % WPG == 0
        self.G = WPC // WPG            # gather groups per core
        self.NPC = WPC * 128           # slots per core
        self.TOT = NC * self.NPC       # total slots
        assert self.TOT // 2 <= 32768  # int16 gather indices
        self.WA = (WPC + 1) // 2       # windows in collective piece A
        self.NA = self.WA * 128
        self.NB = self.NPC - self.NA
        assert self.NA % 2 == 0 and self.NB % 2 == 0
        assert self.N <= self.TOT


FULL = Cfg()

# ---------------------------------------------------------------------------
# host-side graph prep
# ---------------------------------------------------------------------------


def _assign_slots(deg, cfg):
    """LPT-deal nodes into NC*WPC bins of <=128 slots, balancing edge load.
    Returns pos[node] -> global slot position.  Bins within each core are
    then relabeled by descending load so window w has similar load on every
    core — tgt's cross-core max then wastes less padding."""
    import heapq

    nbins = cfg.NC * cfg.WPC
    cap = np.full(nbins, 128, np.int64)
    order = np.argsort(-deg, kind="stable")
    heap = [(0, b) for b in range(nbins)]
    heapq.heapify(heap)
    count = np.zeros(nbins, np.int64)
    load = np.zeros(nbins, np.int64)
    pos = np.empty(cfg.N, np.int64)
    for n in order:
        ld, b = heapq.heappop(heap)
        pos[n] = b * 128 + count[b]
        count[b] += 1
        load[b] = ld + int(deg[n])
        if count[b] < cap[b]:
            heapq.heappush(heap, (load[b], b))
    # per-core bin relabel: rank bins by load (aligning the cross-core max
    # in tgt), then deal ranks round-robin across gather groups so every
    # group carries a similar total load.
    G, WPG = cfg.G, cfg.WPG
    slot_of_rank = np.empty(cfg.WPC, np.int64)
    for r in range(cfg.WPC):
        slot_of_rank[r] = (r % G) * WPG + r // G
    newbin = np.empty(nbins, np.int64)
    for c in range(cfg.NC):
        sl = slice(c * cfg.WPC, (c + 1) * cfg.WPC)
        rank = np.argsort(np.argsort(-load[sl], kind="stable"), kind="stable")
        newbin[sl] = c * cfg.WPC + slot_of_rank[rank]
    b_of = pos // 128
    return newbin[b_of] * 128 + pos % 128


def prep(x, edge_index, cfg):
    """Build per-core input arrays and the static (SPMD-uniform) chunk plan."""
    import ml_dtypes
    bf16 = ml_dtypes.bfloat16
    N, NC, WPC, WPG, G = cfg.N, cfg.NC, cfg.WPC, cfg.WPG, cfg.G
    NPC, DIN, DH = cfg.NPC, cfg.DIN, cfg.DH
    NA, NB = cfg.NA, cfg.NB

    src = edge_index[0].astype(np.int64)
    dst = edge_index[1].astype(np.int64)
    # degree includes the self-loops even though they never hit the gather
    deg = (np.bincount(dst, minlength=N) + 1).astype(np.float32)
    dinv = (1.0 / np.sqrt(deg)).astype(np.float32)

    pos = _assign_slots(deg, cfg)

    # per-core dinv-scaled, transposed x shard + per-slot dinv layouts
    x_shT = np.zeros((NC, DIN, NPC), np.float32)
    dv_slot = np.ones((NC, NPC), np.float32)
    node_of = np.full(cfg.TOT, -1, np.int64)
    node_of[pos] = np.arange(N)
    xs = np.asarray(x) * dinv[:, None]
    for c in range(NC):
        seg = node_of[c * NPC:(c + 1) * NPC]
        m = seg >= 0
        x_shT[c][:, m] = xs[seg[m]].T
        dv_slot[c][m] = dinv[seg[m]]
    dv_s = dv_slot.reshape(NC, WPC, 128).transpose(0, 2, 1).copy()  # [NC,128,WPC]
    # dinv replicated across DH partitions: [NC, DH, WPC*128]
    dvrep = np.broadcast_to(dv_slot[:, None, :], (NC, DH, NPC)).copy()
    dvrep = dvrep.astype(np.float32)

    # edge records (no self-loops in the message stream)
    s_pos = pos[src]
    d_pos = pos[dst]
    # table row = slot; pair-packed 256B elements hold rows (2i, 2i+1)
    half_e = (s_pos & 1).astype(np.int64)
    gidx_e = (s_pos >> 1).astype(np.int64)
    c_e = d_pos // NPC
    w_e = (d_pos % NPC) // 128
    dval_e = (d_pos % 128 + 2).astype(np.float32)

    # bucket edges by (dst core, dst window, src-row parity)
    key_all = (c_e * WPC + w_e) * 2 + half_e
    order_e = np.argsort(key_all, kind="stable")
    ks = key_all[order_e]
    bounds = np.searchsorted(ks, np.arange(NC * WPC * 2 + 1))
    buckets = {}
    for key in range(NC * WPC * 2):
        lo, hi = bounds[key], bounds[key + 1]
        if hi > lo:
            buckets[key] = order_e[lo:hi]

    # per-(window,half) 128-aligned target, equalized across cores
    tgt = np.zeros((WPC, 2), np.int64)
    for w in range(WPC):
        for h in range(2):
            mx = max(len(buckets.get((c * WPC + w) * 2 + h, ()))
                     for c in range(NC))
            tgt[w, h] = int(np.ceil(mx / 128) * 128)

    # per-(group,half) gather segment = concat of member windows' segments
    seglen = np.zeros((G, 2), np.int64)
    for g in range(G):
        for h in range(2):
            seglen[g, h] = tgt[g * WPG:(g + 1) * WPG, h].sum()

    idx_cols = int(sum(seglen[g, h] // 16 for g in range(G) for h in range(2)))
    chunk_tot = int(sum(seglen[g, h] // 128 for g in range(G) for h in range(2)))
    idx_all = np.zeros((NC, 128, idx_cols), np.int16)
    dval_all = np.full((NC, 128, chunk_tot), -1.0, bf16)

    ioff, coff = {}, {}
    io = co = 0
    for g in range(G):
        for h in range(2):
            ioff[(g, h)] = io
            coff[(g, h)] = co
            io += int(seglen[g, h]) // 16
            co += int(seglen[g, h]) // 128
    # chunk column (within dval_all / gather tile) of window w's half-h run
    wcol = np.zeros((WPC, 2), np.int64)
    for g in range(G):
        for h in range(2):
            c0 = coff[(g, h)]
            for wl in range(WPG):
                w = g * WPG + wl
                wcol[w, h] = c0
                c0 += tgt[w, h] // 128

    for c in range(NC):
        for g in range(G):
            for h in range(2):
                n = int(seglen[g, h])
                if n == 0:
                    continue
                gi = np.zeros(n, np.int64)
                dv = np.full(n, -1.0, np.float32)
                p = 0
                for wl in range(WPG):
                    w = g * WPG + wl
                    es = buckets.get((c * WPC + w) * 2 + h, ())
                    ne = len(es)
                    gi[p:p + ne] = gidx_e[es]
                    dv[p:p + ne] = dval_e[es]
                    p += int(tgt[w, h])
                wrapped = gi.reshape(n // 16, 16).T.astype(np.int16)
                idx_all[c, :, ioff[(g, h)]:ioff[(g, h)] + n // 16] = np.tile(
                    wrapped, (8, 1))
                dval_all[c, :, coff[(g, h)]:coff[(g, h)] + n // 128] = (
                    dv.reshape(n // 128, 128).T.astype(bf16))

    plan = dict(tgt=tgt, seglen=seglen, ioff=ioff, coff=coff, wcol=wcol,
                idx_cols=idx_cols, chunk_tot=chunk_tot)
    host = dict(x_shT=x_shT, dv_s=dv_s, dvrep=dvrep,
                idx_all=idx_all, dval_all=dval_all, pos=pos)
    return plan, host


# ---------------------------------------------------------------------------
# device kernel
# ---------------------------------------------------------------------------


def build(cfg, plan):
    import os
    import concourse.bass as bass  # noqa: F401
    import concourse.mybir as mybir
    import concourse.tile as tile
    from concourse import bacc

    STAGE = int(os.environ.get("KERNEL_STAGE", "5"))
    NOCOLL = os.environ.get("KERNEL_NOCOLL", "0") == "1"
    SELFONLY = os.environ.get("KERNEL_SELFONLY", "0") == "1"
    NOSELF = os.environ.get("KERNEL_NOSELF", "0") == "1"

    NC, WPC, WPG, G = cfg.NC, cfg.WPC, cfg.WPG, cfg.G
    NPC, TOT, DIN, DH, DOUT = cfg.NPC, cfg.TOT, cfg.DIN, cfg.DH, cfg.DOUT
    WA, NA, NB = cfg.WA, cfg.NA, cfg.NB
    f32 = mybir.dt.float32
    bf16 = mybir.dt.bfloat16
    tgt, seglen = plan["tgt"], plan["seglen"]
    ioff, coff, wcol = plan["ioff"], plan["coff"], plan["wcol"]
    AF = mybir.ActivationFunctionType
    EH = 64  # element half-stride in bf16 cols (pair-packed 256B elements)
    T2W = 2 * DOUT  # padded layer-2 table row width (bf16 cols)

    nc = bacc.Bacc(
        "TRN2", target_bir_lowering=False, debug=False,
        num_devices=NC, num_swdge_queues=4,
    )

    # I/O
    xT_d = nc.dram_tensor("x_shT", [DIN, NPC], f32, kind="ExternalInput")
    w1_d = nc.dram_tensor("w1", [DIN, DH], f32, kind="ExternalInput")
    w2_d = nc.dram_tensor("w2", [DH, DOUT], f32, kind="ExternalInput")
    b1_d = nc.dram_tensor("b1col", [DH, 1], f32, kind="ExternalInput")
    b2_d = nc.dram_tensor("b2col", [DOUT, 1], f32, kind="ExternalInput")
    aw_d = nc.dram_tensor("awcol", [DOUT, 1], f32, kind="ExternalInput")
    ab_d = nc.dram_tensor("abcol", [128, 1], f32, kind="ExternalInput")
    dv_d = nc.dram_tensor("dv_s", [128, WPC], f32, kind="ExternalInput")
    dvr_d = nc.dram_tensor("dvrep", [DH, NPC], f32, kind="ExternalInput")
    id_d = nc.dram_tensor("ident", [128, 128], f32, kind="ExternalInput")
    gi_d = nc.dram_tensor("giota", [128, 128], bf16, kind="ExternalInput")
    ix_d = nc.dram_tensor("idx_all", [128, plan["idx_cols"]], mybir.dt.int16,
                          kind="ExternalInput")
    dvl_d = nc.dram_tensor("dval_all", [128, plan["chunk_tot"]], bf16,
                           kind="ExternalInput")
    out_d = nc.dram_tensor("out_sh", [NPC, DOUT], f32, kind="ExternalOutput")

    rg = [list(range(NC))]
    qctr = [0]

    with tile.TileContext(nc) as tc:
        with tc.tile_pool(name="const", bufs=1) as cpool:
            def load(dram, shape, dt=f32):
                t = cpool.tile(shape, dt, tag=dram.name, name=dram.name + "_s")
                nc.sync.dma_start(t[:], dram.ap())
                return t

            w1_s = load(w1_d, [DIN, DH])
            w2_s = load(w2_d, [DH, DOUT])
            b1_s = load(b1_d, [DH, 1])
            b2_s = load(b2_d, [DOUT, 1])
            aw_s = load(aw_d, [DOUT, 1])
            ab_s = load(ab_d, [128, 1])
            dv_s = load(dv_d, [128, WPC])
            dvr_s = load(dvr_d, [DH, NPC])
            id_s = load(id_d, [128, 128])
            gi_s = load(gi_d, [128, 128], bf16)
            ix_s = load(ix_d, [128, plan["idx_cols"]], mybir.dt.int16)
            dvl_s = load(dvl_d, [128, plan["chunk_tot"]], bf16)

            # SBUF-resident local table shards (self-loop matmul operands)
            idb_s = cpool.tile([128, 128], bf16, tag="idb", name="idb")
            t1_sb = cpool.tile([128, WPC * DH], bf16, tag="t1sb", name="t1sb")
            t2_sb = cpool.tile([128, WPC * T2W], bf16, tag="t2sb", name="t2sb")
            nc.vector.memset(t2_sb[:], 0.0)
            nc.vector.tensor_copy(idb_s[:], id_s[:])

            with tc.tile_pool(name="dram", bufs=1, space="DRAM") as dpool:
                t1_shard = dpool.tile([NPC, DH], bf16, tag="t1s", name="t1s")
                t1_full = dpool.tile([TOT, DH], bf16, tag="t1f", name="t1f",
                                     addr_space="Shared")
                t2_shard = dpool.tile([NPC, T2W], bf16, tag="t2s", name="t2s")
                t2_full = dpool.tile([TOT, T2W], bf16, tag="t2f", name="t2f",
                                     addr_space="Shared")

                def allgather(shard, full):
                    if NOCOLL:
                        nc.sync.dma_start(full[0:NPC, :], shard[:])
                    else:
                        nc.gpsimd.collective_compute(
                            "AllGather", mybir.AluOpType.bypass,
                            replica_groups=rg, ins=[shard[:]],
                            outs=[full[:]],
                        )

                # ---- phase 1: T1 = (dinv .* x) @ W1 per window
                with (
                    tc.tile_pool(name="tf_in", bufs=1) as pin,
                    tc.tile_pool(name="tf_ps", bufs=3, space="PSUM") as pps,
                ):
                    xt_all = pin.tile([DIN, NPC], f32, tag="xt", name="xt")
                    nc.sync.dma_start(xt_all[:], xT_d.ap())
                    for w in range(WPC):
                        hp = pps.tile([128, DH], f32, tag="hp", name="hp")
                        nc.tensor.matmul(hp[:],
                                         lhsT=xt_all[:, w * 128:(w + 1) * 128],
                                         rhs=w1_s[:],
                                         start=True, stop=True)
                        nc.scalar.activation(
                            t1_sb[:, w * DH:(w + 1) * DH], hp[:],
                            func=AF.Copy)
                    # one wrap-DMA for the whole shard (shorter critical
                    # path into the AllGather than 49 window writes)
                    nc.sync.dma_start(
                        t1_shard[:].rearrange("(w p) d -> p w d", p=128),
                        t1_sb[:].rearrange("p (w d) -> p w d", d=DH))
                    allgather(t1_shard, t1_full)

                if STAGE == 1:
                    with tc.tile_pool(name="dbg", bufs=2) as dbg:
                        for w in range(WPC):
                            d = dbg.tile([128, DOUT], bf16, name="d")
                            nc.sync.dma_start(
                                d[:], t1_full[w * 128:(w + 1) * 128, :DOUT])
                            d2 = dbg.tile([128, DOUT], f32, name="d2")
                            nc.vector.tensor_copy(d2[:], d[:])
                            nc.sync.dma_start(
                                out_d.ap()[w * 128:(w + 1) * 128, :], d2[:])
                    nc.compile()
                    return nc

                # ---- aggregation: gather + one-hot matmul segment sums,
                # accumulated d-major ([ncols, 128] PSUM per window)
                def aggregate(full, ncols, self_sb, self_w, flush_fn,
                              ag_hook=None):
                    fv = full.rearrange("(a b) d -> a (b d)", b=2)
                    with (
                        tc.tile_pool(name="gpool", bufs=4) as gp,
                        tc.tile_pool(name="spool", bufs=3) as sp,
                        tc.tile_pool(name="apsum", bufs=4, space="PSUM") as aps,
                    ):
                        pend = []
                        for g in range(G):
                            gts = {}
                            for h in range(2):
                                n = int(seglen[g, h])
                                if n == 0:
                                    continue
                                nch = n // 128
                                gt = gp.tile([128, nch * 128], bf16,
                                             tag=f"g{h}", name=f"gt{h}")
                                io = ioff[(g, h)]
                                # many small sub-gathers: each binds ~one
                                # SDMA engine, so concurrency sets bandwidth
                                NSUB = 8
                                step = (n // (NSUB * 128)) * 128
                                cuts = [i * step for i in range(NSUB)] + [n]
                                for o0, o1 in zip(cuts[:-1], cuts[1:]):
                                    nn = o1 - o0
                                    if nn == 0:
                                        continue
                                    nc.gpsimd.dma_gather(
                                        out_ap=gt[:, o0:o0 + nn].rearrange(
                                            "p (c d) -> p c d", d=128),
                                        in_ap=fv[:, :],
                                        idxs_ap=ix_s[:, io + o0 // 16:
                                                     io + o1 // 16],
                                        num_idxs=nn, num_idxs_reg=nn,
                                        elem_size=128,
                                        queue_num=qctr[0] % 4,
                                        single_packet=False,
                                    )
                                    qctr[0] += 1
                                gts[h] = gt
                            if STAGE == 6 and g == 0:
                                w = 0
                                S6 = {}
                                for h in range(2):
                                    k = int(tgt[w, h]) // 128
                                    S = sp.tile([128, k * 128], bf16,
                                                tag=f"S{h}", name=f"S{h}")
                                    c0 = int(wcol[w, h])
                                    dvb = dvl_s[:, c0:c0 + k].unsqueeze(2) \
                                        .broadcast_to([128, k, 128])
                                    gib = gi_s[:].unsqueeze(1) \
                                        .broadcast_to([128, k, 128])
                                    nc.vector.tensor_tensor(
                                        out=S[:].rearrange(
                                            "p (k j) -> p k j", k=k),
                                        in0=dvb, in1=gib,
                                        op=mybir.AluOpType.is_equal)
                                    S6[h] = S
                                with tc.tile_pool(name="dbg6", bufs=1) as dbg:
                                    dumps = [gts[0][:, 0:128],
                                             gts[1][:, 0:128],
                                             S6[0][:, 0:128],
                                             S6[1][:, 0:128]]
                                    for r, srcap in enumerate(dumps):
                                        d2 = dbg.tile([128, 128], f32,
                                                      name=f"d6_{r}")
                                        nc.vector.tensor_copy(d2[:], srcap)
                                        nc.sync.dma_start(
                                            out_d.ap()[r * 128:(r + 1) * 128,
                                                       :],
                                            d2[:, :DOUT])
                                nc.compile()
                                return True
                            for wl in range(WPG):
                                w = g * WPG + wl
                                # bulk one-hot S build: one op per (w, half)
                                Ss = {}
                                for h in range(2):
                                    k = int(tgt[w, h]) // 128
                                    if k == 0:
                                        continue
                                    S = sp.tile([128, k * 128], bf16,
                                                tag=f"S{h}", name=f"S{h}")
                                    c0 = int(wcol[w, h])
                                    dvb = dvl_s[:, c0:c0 + k].unsqueeze(2) \
                                        .broadcast_to([128, k, 128])
                                    gib = gi_s[:].unsqueeze(1) \
                                        .broadcast_to([128, k, 128])
                                    nc.vector.tensor_tensor(
                                        out=S[:].rearrange(
                                            "p (k j) -> p k j", k=k),
                                        in0=dvb, in1=gib,
                                        op=mybir.AluOpType.is_equal)
                                    Ss[h] = S
                                ps = aps.tile([ncols, 128], f32, tag="agg",
                                              name="agg")
                                chunks = (
                                    [(0, k) for k in range(int(tgt[w, 0]) // 128)]
                                    + [(1, k) for k in range(int(tgt[w, 1]) // 128)]
                                )
                                if SELFONLY:
                                    chunks = []
                                # self-loop chunk first: diag(dinv) selection
                                if not NOSELF:
                                    nc.tensor.matmul(
                                        ps[:],
                                        lhsT=self_sb[:, w * self_w:
                                                     w * self_w + ncols],
                                        rhs=idb_s[:],
                                        start=True, stop=(len(chunks) == 0))
                                for j, (h, k) in enumerate(chunks):
                                    tcol = int(wcol[w, h] - coff[(g, h)]) + k
                                    base = tcol * 128 + h * EH
                                    nc.tensor.matmul(
                                        ps[:],
                                        lhsT=gts[h][:, base:base + ncols],
                                        rhs=Ss[h][:, k * 128:(k + 1) * 128],
                                        start=(NOSELF and j == 0),
                                        stop=(j == len(chunks) - 1),
                                    )
                                if len(pend) == 2:
                                    pw, pps = pend.pop(0)
                                    flush_fn(pw, pps)
                                    if ag_hook is not None:
                                        ag_hook(pw)
                                pend.append((w, ps))
                        for pw, pps in pend:
                            flush_fn(pw, pps)
                            if ag_hook is not None:
                                ag_hook(pw)

                # ---- layer-1 flush: h1=relu(dv*agg+b1); T2 = (dv.*h1)@W2
                with (
                    tc.tile_pool(name="fl1_sb", bufs=4) as fsb1,
                    tc.tile_pool(name="fl1_ps", bufs=2, space="PSUM") as fps1,
                ):
                    def flush1_dbg(w, ps):
                        # STAGE 2: dump raw agg (first DOUT rows, transposed)
                        c1 = fsb1.tile([DH, 128], f32, tag="a1", name="a1")
                        nc.vector.tensor_copy(c1[:], ps[:])
                        tp = fps1.tile([128, DOUT], f32, tag="t2p", name="tp")
                        nc.tensor.matmul(tp[:], lhsT=c1[:],
                                         rhs=id_s[0:DH, 0:DOUT],
                                         start=True, stop=True)
                        o = fsb1.tile([128, DOUT], f32, tag="a2", name="o")
                        nc.scalar.activation(o[:], tp[:], func=AF.Copy)
                        nc.sync.dma_start(
                            out_d.ap()[w * 128:(w + 1) * 128, :], o[:])

                    def flush1(w, ps):
                        a1 = fsb1.tile([DH, 128], f32, tag="a1", name="a1")
                        nc.vector.tensor_tensor(
                            out=a1[:], in0=ps[:],
                            in1=dvr_s[:, w * 128:(w + 1) * 128],
                            op=mybir.AluOpType.mult)
                        a2 = fsb1.tile([DH, 128], f32, tag="a2", name="a2")
                        nc.scalar.activation(a2[:], a1[:], func=AF.Relu,
                                             bias=b1_s[:, 0:1])
                        t2p = fps1.tile([128, DOUT], f32, tag="t2p",
                                        name="t2p")
                        nc.tensor.matmul(t2p[:], lhsT=a2[:], rhs=w2_s[:],
                                         start=True, stop=True)
                        # outer dinv rides the f32->bf16 cast
                        nc.scalar.activation(
                            t2_sb[:, w * T2W:w * T2W + DOUT], t2p[:],
                            func=AF.Copy, scale=dv_s[:, w:w + 1])
                        nc.sync.dma_start(
                            t2_shard[w * 128:(w + 1) * 128, :],
                            t2_sb[:, w * T2W:(w + 1) * T2W])

                    def ag2_hook(w):
                        if w == WPC - 1:
                            allgather(t2_shard, t2_full)

                    if STAGE == 2:
                        aggregate(t1_full[:], DH, t1_sb, DH, flush1_dbg)
                        nc.compile()
                        return nc

                    if STAGE == 6:
                        aggregate(t1_full[:], DH, t1_sb, DH, flush1)
                        return nc

                    aggregate(t1_full[:], DH, t1_sb, DH, flush1, ag2_hook)

                if STAGE == 3:
                    with tc.tile_pool(name="dbg", bufs=2) as dbg:
                        for w in range(WPC):
                            d = dbg.tile([128, DOUT], bf16, name="d")
                            nc.sync.dma_start(
                                d[:],
                                t2_full[w * 128:(w + 1) * 128, :DOUT])
                            d2 = dbg.tile([128, DOUT], f32, name="d2")
                            nc.vector.tensor_copy(d2[:], d[:])
                            nc.sync.dma_start(
                                out_d.ap()[w * 128:(w + 1) * 128, :],
                                d2[:])
                    nc.compile()
                    return nc

                # ---- layer-2 flush: h2 + attention gate -> out
                with (
                    tc.tile_pool(name="fl2_sb", bufs=4) as fsb2,
                    tc.tile_pool(name="fl2_ps", bufs=2, space="PSUM") as fps2,
                ):
                    def flush2(w, ps):
                        a1 = fsb2.tile([DOUT, 128], f32, tag="f2a1",
                                       name="f2a1")
                        nc.vector.tensor_tensor(
                            out=a1[:], in0=ps[:],
                            in1=dvr_s[0:DOUT, w * 128:(w + 1) * 128],
                            op=mybir.AluOpType.mult)
                        h2f = fsb2.tile([DOUT, 128], f32, tag="f2h",
                                        name="f2h")
                        nc.scalar.activation(h2f[:], a1[:], func=AF.Relu,
                                             bias=b2_s[:, 0:1])
                        app = fps2.tile([128, 1], f32, tag="f2ap",
                                        name="f2ap")
                        nc.tensor.matmul(app[:], lhsT=h2f[:], rhs=aw_s[:],
                                         start=True, stop=True)
                        at = fsb2.tile([128, 1], f32, tag="f2at", name="f2at")
                        nc.scalar.activation(at[:], app[:], func=AF.Sigmoid,
                                             bias=ab_s[:, 0:1])
                        op = fps2.tile([128, DOUT], f32, tag="f2op",
                                       name="f2op")
                        nc.tensor.matmul(op[:], lhsT=h2f[:],
                                         rhs=id_s[0:DOUT, 0:DOUT],
                                         start=True, stop=True)
                        o = fsb2.tile([128, DOUT], f32, tag="f2o", name="f2o")
                        nc.scalar.activation(o[:], op[:], func=AF.Copy,
                                             scale=at[:, 0:1])
                        nc.sync.dma_start(
                            out_d.ap()[w * 128:(w + 1) * 128, :], o[:])

                    aggregate(t2_full[:], DOUT, t2_sb, T2W, flush2)

    nc.compile()
    return nc


# ---------------------------------------------------------------------------
# entry point
# ---------------------------------------------------------------------------


def _make_in_maps(cfg, host, W1, b1, W2, b2, attn_w, attn_b):
    import ml_dtypes
    NC = cfg.NC
    bf16 = ml_dtypes.bfloat16
    ident = np.eye(128, dtype=np.float32)
    giota = np.tile(np.arange(2, 130, dtype=np.float32), (128, 1)).astype(bf16)
    in_maps = []
    for c in range(NC):
        in_maps.append({
            "x_shT": host["x_shT"][c],
            "w1": np.asarray(W1, np.float32),
            "w2": np.asarray(W2, np.float32),
            "b1col": np.asarray(b1, np.float32).reshape(-1, 1),
            "b2col": np.asarray(b2, np.float32).reshape(-1, 1),
            "awcol": np.asarray(attn_w, np.float32).reshape(-1, 1),
            "abcol": np.full((128, 1),
                             np.asarray(attn_b, np.float32).reshape(-1)[0],
                             np.float32),
            "dv_s": host["dv_s"][c],
            "dvrep": host["dvrep"][c],
            "ident": ident,
            "giota": giota,
            "idx_all": host["idx_all"][c],
            "dval_all": host["dval_all"][c],
        })
    return in_maps


def run(x, edge_index, W1, b1, W2, b2, attn_w, attn_b, cfg=None,
        backend="hw", trace=False):
    cfg = cfg or FULL
    plan, host = prep(x, edge_index, cfg)
    nc = build(cfg, plan)
    in_maps = _make_in_maps(cfg, host, W1, b1, W2, b2, attn_w, attn_b)

    if backend == "sim":
        from concourse.bass_interp import MultiCoreSim
        sim = MultiCoreSim(nc, num_cores=cfg.NC, trace=False)
        for c, core in enumerate(sim.cores.values()):
            for name, arr in in_maps[c].items():
                core.tensor(name)[:] = arr
        sim.simulate()
        outs = [core.tensor("out_sh").copy() for core in sim.cores.values()]
        exec_ns = None
    else:
        from concourse import bass_utils
        from concourse.bass_interp import get_hw_module
        old = nc.m
        nc.m = get_hw_module(nc.m)
        try:
            res = bass_utils.run_bass_kernel_spmd(
                nc, in_maps, core_ids=list(range(cfg.NC)), trace=trace)
        finally:
            nc.m = old
        outs = [res.results[c]["out_sh"] for c in range(cfg.NC)]
        exec_ns = res.exec_time_ns

    full = np.concatenate(outs, axis=0)  # [TOT, DOUT] in slot order
    out = full[host["pos"]]              # unpermute -> [N, DOUT]
    return np.ascontiguousarray(out), exec_ns


def kernel(x, edge_index, W1, b1, W2, b2, attn_w, attn_b):
    x = np.asarray(x, np.float32)
    edge_index = np.asarray(edge_index)
    out, _ = run(x, edge_index, W1, b1, W2, b2, attn_w, attn_b,
                 cfg=FULL, backend="hw", trace=False)
    return out
